# revision 29
# baseline (speedup 1.0000x reference)
"""Causal self-attention (B=4, T=4096, D=1024, fp32) on 8 trn2 NeuronCores.

Sharding: 2 cores per batch. Within a batch, core h in {0,1} owns the
key blocks of parity h (128-wide blocks at global positions 2j+h). Each
core computes, for ALL queries of its batch, the unnormalized partial
attention output restricted to its own keys, plus the partial softmax
denominators. Host merge per query:
    out[q] = (o0[:,q] + o1[:,q]) / (denom0[q] + denom1[q]) / 32.

Optimizations over the plain formulation:

1. W_o folded into the V projection on the host:
   (att @ (x W_v^T)) W_o^T == att @ (x (W_o W_v)^T), so the device
   projects v' = x (W_o W_v)^T once; the attention-weighted sum of v'
   IS the (unnormalized) output. Deletes the on-device W_o GEMM (which
   was also duplicated across the pair).

2. Everything heavy runs in fp8e4 with perf_mode=DoubleRow (two
   128-deep contraction slabs per instruction = 2x the bf16 rate):
   the Q/K/V' projections (host supplies fp8 x and 32x-scaled fp8
   weights; the 32x cancels exactly: scores get 2^-15 folded into the
   Exp activation scale, v' is divided by 32 in the host merge) and the
   attention scores / att@v' / denominator GEMMs. fp32 PSUM throughout.

3. Query tile 0 (global queries 0..511) stays bf16 end to end (its own
   small bf16 projections of the first 512 queries / 256 parity keys):
   for tiny softmax supports the fp8 noise does not average out.

4. The pair-wise Q^T AllGather is fp8 (half the bytes), preceded by a
   small early bf16 gather of the first 512 query columns for tile 0.
   Both hide under the projection phase.

Scores accumulate unscaled; softmax scale 1/sqrt(D) * 1/1024 (weight
scaling) = 2^-15 and a range bias -2.5 (keeps exp inside e4m3 range,
cancels in the normalization) fold into the Exp activation. The causal
mask is additive (-4096 on raw x32-scaled scores) applied on PSUM
before exp.
"""

import sys

if "/opt/trn_rl_repo" not in sys.path:
    sys.path.insert(0, "/opt/trn_rl_repo")

import numpy as np
import ml_dtypes

BF16 = ml_dtypes.bfloat16
F8 = ml_dtypes.float8_e4m3fn

D = 1024
P = 128          # partition / contraction block
DB = D // P      # 8 d-blocks

EXP_BIAS = -3.5      # exp(score + EXP_BIAS); cancels in the normalization
EXP_SCALE = 2.0 ** -15  # 1/sqrt(D) softmax scale * (1/32)^2 weight scaling
MASK_VAL = -4096.0 * 1024.0  # additive mask on RAW x32-scaled scores

_PROGRAM_CACHE = {}


def build_program(T, TQ):
    """Build + compile the single-core SPMD program. Returns the Bacc."""
    import concourse.mybir as mybir
    import concourse.tile as tile
    from concourse import bacc

    bf = mybir.dt.bfloat16
    f8 = mybir.dt.float8e4
    f32 = mybir.dt.float32
    f16 = mybir.dt.float16
    DR = mybir.MatmulPerfMode.DoubleRow

    NT = T // TQ             # q-tiles per core (8)
    NM = TQ // 256           # diagonal (masked) local key blocks per q-tile
    TKV = T // 2             # parity keys per core (2048)
    NKB = TKV // P           # local key blocks (16)
    KV_TT = 512              # token tile for the projection phases
    NKVT = TKV // KV_TT      # 4
    TH = T // 2              # this core's query half
    TB0 = 512                # bf16 query prefix (tile 0)
    KB0 = NM * P             # bf16 local key prefix (256)

    nc = bacc.Bacc("TRN2", target_bir_lowering=False, debug=False, num_devices=8)

    xq8 = nc.dram_tensor("xq8", [D, TH], f8, kind="ExternalInput")
    xqb = nc.dram_tensor("xqb", [D, TB0], bf, kind="ExternalInput")
    xkv8 = nc.dram_tensor("xkv8", [D, TKV], f8, kind="ExternalInput")
    xkvb = nc.dram_tensor("xkvb", [D, KB0], bf, kind="ExternalInput")
    wq8 = nc.dram_tensor("wq8", [D, D], f8, kind="ExternalInput")
    wk8 = nc.dram_tensor("wk8", [D, D], f8, kind="ExternalInput")
    wv8 = nc.dram_tensor("wv8", [D, D], f8, kind="ExternalInput")
    wqb = nc.dram_tensor("wqb", [D, D], bf, kind="ExternalInput")
    wkb = nc.dram_tensor("wkb", [D, D], bf, kind="ExternalInput")
    wvb = nc.dram_tensor("wvb", [D, D], bf, kind="ExternalInput")
    mask = nc.dram_tensor("mask", [NM, P, TQ], bf, kind="ExternalInput")
    outT = nc.dram_tensor("outT", [D, T], f16, kind="ExternalOutput")
    denom = nc.dram_tensor("denom", [NT, TQ], f32, kind="ExternalOutput")

    xq8_r = xq8.rearrange("(po pi) t -> pi po t", pi=P)
    xqb_r = xqb.rearrange("(po pi) t -> pi po t", pi=P)
    xkv8_r = xkv8.rearrange("(po pi) t -> pi po t", pi=P)
    xkvb_r = xkvb.rearrange("(po pi) t -> pi po t", pi=P)
    wq8_r = wq8.rearrange("(po pi) f -> pi po f", pi=P)
    wk8_r = wk8.rearrange("(po pi) f -> pi po f", pi=P)
    wv8_r = wv8.rearrange("(po pi) f -> pi po f", pi=P)
    wqb_r = wqb.rearrange("(po pi) f -> pi po f", pi=P)
    wkb_r = wkb.rearrange("(po pi) f -> pi po f", pi=P)
    wvb_r = wvb.rearrange("(po pi) f -> pi po f", pi=P)
    outT_r = outT.rearrange("(po pi) t -> pi po t", pi=P)

    with tile.TileContext(nc) as tc:
        with tc.tile_pool(name="res", bufs=1) as res, \
             tc.tile_pool(name="wts", bufs=1) as wts, \
             tc.tile_pool(name="dram", bufs=1, space="DRAM") as dram:
            # Persistent SBUF: fp8 K^T (d-major) and V' (token-major) for
            # the DoubleRow path; bf16 copies of the first NM key blocks
            # for the bf16 tile-0 path.
            kT8_sb = res.tile([P, DB, TKV], f8)
            v8_sb = res.tile([P, NKB, D], f8)
            kTb_sb = res.tile([P, DB, KB0], bf)
            vb_sb = res.tile([P, NM, D], bf)
            mask_sb = res.tile([P, NM, TQ], bf)
            ones_b = res.tile([P, 1], bf)
            ones_8 = res.tile([P, 2, 16], f8)
            ebias_sb = res.tile([P, 1], f32)
            nc.vector.memset(ones_b[:], 1.0)
            nc.vector.memset(ones_8[:], 1.0)
            nc.vector.memset(ebias_sb[:], EXP_BIAS)

            # All weights + small bf16 x slices loaded upfront on the sync
            # queue, ordered by first use. Intermediate q write-outs go on
            # the scalar queue and B outputs on the gpsimd queue, so this
            # queue only ever carries input loads (no head-of-line blocks).
            wq8_sb = wts.tile([P, DB, D], f8)
            wqb_sb = wts.tile([P, DB, D], bf)
            wk8_sb = wts.tile([P, DB, D], f8)
            wv8_sb = wts.tile([P, DB, D], f8)
            wkb_sb = wts.tile([P, DB, D], bf)
            wvb_sb = wts.tile([P, DB, D], bf)
            xqb_t = wts.tile([P, DB, TB0], bf)
            xkvb_t = wts.tile([P, DB, KB0], bf)
            qTb_sb = wts.tile([P, DB, TB0], bf)  # tile-0 Q^T, stays in SBUF
            # Latency-critical A0 stream goes first on sync; the bulk
            # loads for later phases are emitted after the A0-fp8 loop.
            nc.sync.dma_start(wq8_sb[:], wq8_r[:])

            # Pair-gathered fp8 Q^T. Rows [0:D] = first query half (rank
            # 2b), rows [D:2D] = second half (rank 2b+1).
            q8_local = dram.tile([D, TH], f8)
            q8_full = dram.tile([2 * D, TH], f8)

            # ---- Phase A0: Q projection of this core's query half ----
            with tc.tile_pool(name="pq_sb", bufs=2) as pq_sb, \
                 tc.tile_pool(name="pq_ps", bufs=2, space="PSUM") as pq_ps:
                # fp8 DoubleRow projection of the full query half first, so
                # the big gather fires as early as possible.
                q8_local_r = q8_local.rearrange("(po pi) t -> pi po t", pi=P)
                for it in range(TH // KV_TT):
                    xq = pq_sb.tile([P, DB, KV_TT], f8, tag="xq")
                    for po in range(DB):
                        nc.sync.dma_start(
                            xq[:, po, :],
                            xq8_r[:, po, it * KV_TT:(it + 1) * KV_TT])
                    qstage = pq_sb.tile([P, DB, KV_TT], f8, tag="qstage")
                    for do in range(DB):
                        qp = pq_ps.tile([P, KV_TT], f32, tag="qp")
                        for dd in range(DB // 2):
                            nc.tensor.matmul(
                                qp[:],
                                wq8_sb[:, 2 * dd:2 * dd + 2, do * P:(do + 1) * P],
                                xq[:, 2 * dd:2 * dd + 2, :],
                                start=(dd == 0), stop=(dd == DB // 2 - 1),
                                perf_mode=DR)
                        nc.vector.tensor_copy(qstage[:, do, :], qp[:])
                    for po in range(DB):
                        nc.gpsimd.dma_start(
                            q8_local_r[:, po, it * KV_TT:(it + 1) * KV_TT],
                            qstage[:, po, :])
                nc.gpsimd.collective_compute(
                    "AllGather",
                    mybir.AluOpType.bypass,
                    replica_groups=[[0, 1], [2, 3], [4, 5], [6, 7]],
                    ins=[q8_local[:]],
                    outs=[q8_full[:]],
                )

                # Bulk loads for the remaining phases, in first-use order,
                # queued behind the A0-fp8 x stream on sync.
                nc.sync.dma_start(wqb_sb[:], wqb_r[:])
                nc.sync.dma_start(xqb_t[:], xqb_r[:])
                nc.sync.dma_start(wkb_sb[:], wkb_r[:])
                nc.sync.dma_start(xkvb_t[:], xkvb_r[:])
                nc.sync.dma_start(wk8_sb[:], wk8_r[:])
                nc.sync.dma_start(wv8_sb[:], wv8_r[:])
                nc.sync.dma_start(wvb_sb[:], wvb_r[:])
                nc.sync.dma_start(mask_sb[:], mask.rearrange("m p t -> p m t"))

                # bf16 projection of the GLOBAL first TB0 queries (the host
                # hands both pair cores the same xqb slice, so no gather is
                # needed); result stays resident in SBUF for tile 0.
                for do in range(DB):
                    qp = pq_ps.tile([P, TB0], f32, tag="qp")
                    for di in range(DB):
                        nc.tensor.matmul(
                            qp[:],
                            wqb_sb[:, di, do * P:(do + 1) * P],
                            xqb_t[:, di, :],
                            start=(di == 0), stop=(di == DB - 1))
                    nc.vector.tensor_copy(qTb_sb[:, do, :], qp[:])

            # ---- Phase A: K/V' projection of the parity keys ----
            with tc.tile_pool(name="pa_sb", bufs=2) as pa_sb, \
                 tc.tile_pool(name="pa_ps", bufs=2, space="PSUM") as pa_ps:
                # bf16 dup of the first KB0 parity keys (tile-0 path).
                for do in range(DB):
                    kps = pa_ps.tile([P, KB0], f32, tag="kpsb")
                    for di in range(DB):
                        nc.tensor.matmul(
                            kps[:],
                            wkb_sb[:, di, do * P:(do + 1) * P],
                            xkvb_t[:, di, :],
                            start=(di == 0), stop=(di == DB - 1))
                    nc.vector.tensor_copy(kTb_sb[:, do, :], kps[:])
                for tb in range(NM):
                    for dh in range(D // 512):
                        vps = pa_ps.tile([P, 512], f32, tag="vpsb")
                        for di in range(DB):
                            nc.tensor.matmul(
                                vps[:],
                                xkvb_t[:, di, tb * P:(tb + 1) * P],
                                wvb_sb[:, di, dh * 512:(dh + 1) * 512],
                                start=(di == 0), stop=(di == DB - 1))
                        nc.vector.tensor_copy(
                            vb_sb[:, tb, dh * 512:(dh + 1) * 512], vps[:])

                # fp8 DoubleRow projections of all parity keys.
                for tt in range(NKVT):
                    xkv = pa_sb.tile([P, DB, KV_TT], f8, tag="xkv")
                    nc.sync.dma_start(
                        xkv[:], xkv8_r[:, :, tt * KV_TT:(tt + 1) * KV_TT])
                    # K^T[dout, tok] += W_k^T[din, dout].T @ x^T[din, tok]
                    for do in range(DB):
                        kps = pa_ps.tile([P, KV_TT], f32, tag="kps")
                        for dd in range(DB // 2):
                            nc.tensor.matmul(
                                kps[:],
                                wk8_sb[:, 2 * dd:2 * dd + 2, do * P:(do + 1) * P],
                                xkv[:, 2 * dd:2 * dd + 2, :],
                                start=(dd == 0), stop=(dd == DB // 2 - 1),
                                perf_mode=DR)
                        nc.vector.tensor_copy(
                            kT8_sb[:, do, tt * KV_TT:(tt + 1) * KV_TT], kps[:])
                    # V'[tok, dout] += x^T[din, tok].T @ W_vo^T[din, dout]
                    for tb in range(KV_TT // P):
                        for dh in range(D // 512):
                            vps = pa_ps.tile([P, 512], f32, tag="vps")
                            for dd in range(DB // 2):
                                nc.tensor.matmul(
                                    vps[:],
                                    xkv[:, 2 * dd:2 * dd + 2,
                                        tb * P:(tb + 1) * P],
                                    wv8_sb[:, 2 * dd:2 * dd + 2,
                                           dh * 512:(dh + 1) * 512],
                                    start=(dd == 0), stop=(dd == DB // 2 - 1),
                                    perf_mode=DR)
                            jb = tt * (KV_TT // P) + tb
                            nc.vector.tensor_copy(
                                v8_sb[:, jb, dh * 512:(dh + 1) * 512], vps[:])

            # ---- Phase B: per q-tile attention (already the output) ----
            q8_full_r = q8_full.rearrange("(ho po pi) t -> pi ho po t",
                                          pi=P, po=DB)
            with tc.tile_pool(name="pb_sb", bufs=2) as pb_sb, \
                 tc.tile_pool(name="pb_pan", bufs=2) as pb_pan, \
                 tc.tile_pool(name="pb_out", bufs=4) as pb_out, \
                 tc.tile_pool(name="s_ps", bufs=3, space="PSUM") as s_ps, \
                 tc.tile_pool(name="y_ps", bufs=3, space="PSUM") as y_ps, \
                 tc.tile_pool(name="d_ps", bufs=1, space="PSUM") as d_ps:
                for i in range(NT):
                    nkb = (i + 1) * NM  # local key blocks for this q-tile
                    q0 = i * TQ
                    ho = q0 // TH       # which gathered half holds this tile
                    qh = q0 - ho * TH
                    f8_tile = i > 0     # tile 0 stays bf16

                    if f8_tile:
                        qT8 = pb_sb.tile([P, DB, TQ], f8, tag="qT8")
                        for po in range(DB):
                            nc.sync.dma_start(
                                qT8[:, po, :],
                                q8_full_r[:, ho, po, qh:qh + TQ])
                    else:
                        qTb = qTb_sb  # resident since phase A0

                    # S^T blocks -> mask -> exp -> panel
                    if f8_tile:
                        panel = pb_pan.tile([P, NT * NM, TQ], f8, tag="pan8")
                    else:
                        panel = pb_pan.tile([P, NM, TQ], bf, tag="panb")
                    for j in range(nkb):
                        sps = s_ps.tile([P, TQ], f32, tag="s")
                        if f8_tile:
                            for dd in range(DB // 2):
                                nc.tensor.matmul(
                                    sps[:],
                                    kT8_sb[:, 2 * dd:2 * dd + 2,
                                           j * P:(j + 1) * P],
                                    qT8[:, 2 * dd:2 * dd + 2, :],
                                    start=(dd == 0), stop=(dd == DB // 2 - 1),
                                    perf_mode=DR)
                        else:
                            for di in range(DB):
                                nc.tensor.matmul(
                                    sps[:],
                                    kTb_sb[:, di, j * P:(j + 1) * P],
                                    qTb[:, di, :],
                                    start=(di == 0), stop=(di == DB - 1))
                        if j >= nkb - NM:
                            m = j - (nkb - NM)
                            nc.vector.tensor_add(
                                out=sps[:], in0=sps[:], in1=mask_sb[:, m, :])
                        nc.scalar.activation(
                            panel[:, j, :], sps[:],
                            mybir.ActivationFunctionType.Exp,
                            bias=ebias_sb[:], scale=EXP_SCALE)

                    # Denominators after all panels (keeps the tensor queue
                    # from stalling on the exp producer mid-stream).
                    dps = d_ps.tile([1, TQ], f32, tag="den")
                    if f8_tile:
                        for jj in range(nkb // 2):
                            nc.tensor.matmul(
                                dps[:], ones_8[:, :, 0:1],
                                panel[:, 2 * jj:2 * jj + 2, :],
                                start=(jj == 0), stop=(jj == nkb // 2 - 1),
                                perf_mode=DR)
                    else:
                        for j in range(nkb):
                            nc.tensor.matmul(
                                dps[:], ones_b[:], panel[:, j, :],
                                start=(j == 0), stop=(j == nkb - 1))
                    dstage = pb_sb.tile([1, TQ], f32, tag="dstage")
                    nc.vector.tensor_copy(dstage[:], dps[:])
                    nc.gpsimd.dma_start(denom[i:i + 1, :], dstage[0:1, :])

                    # outT[dout, q] += V'[k, dout].T @ expS^T[k, q]
                    for do in range(DB):
                        yps = y_ps.tile([P, TQ], f32, tag="y")
                        if f8_tile:
                            for jj in range(nkb // 2):
                                nc.tensor.matmul(
                                    yps[:],
                                    v8_sb[:, 2 * jj:2 * jj + 2,
                                          do * P:(do + 1) * P],
                                    panel[:, 2 * jj:2 * jj + 2, :],
                                    start=(jj == 0), stop=(jj == nkb // 2 - 1),
                                    perf_mode=DR)
                        else:
                            for j in range(nkb):
                                nc.tensor.matmul(
                                    yps[:],
                                    vb_sb[:, j, do * P:(do + 1) * P],
                                    panel[:, j, :],
                                    start=(j == 0), stop=(j == nkb - 1))
                        ostage = pb_out.tile([P, TQ], f16, tag="ostage")
                        nc.vector.tensor_copy(ostage[:], yps[:])
                        nc.gpsimd.dma_start(
                            outT_r[:, do, q0:q0 + TQ], ostage[:])

    nc.compile()
    return nc


def _prepare_core_inputs(x, W_q, W_k, W_v, W_o, T, TQ):
    """Host-side shard prep. Returns list of 8 in_maps."""
    B = x.shape[0]

    # Fold W_o into the V projection; scale all weights by 32 so their
    # fp8 casts stay out of e4m3's subnormal range (exactly compensated
    # via EXP_SCALE and the /32 in the merge).
    w_vo = np.asarray(W_o, np.float32) @ np.asarray(W_v, np.float32)
    wqT = np.ascontiguousarray(W_q.T) * np.float32(32.0)
    wkT = np.ascontiguousarray(W_k.T) * np.float32(32.0)
    wvT = np.ascontiguousarray(w_vo.T) * np.float32(32.0)
    w8 = {n: w.astype(F8) for n, w in (("wq8", wqT), ("wk8", wkT), ("wv8", wvT))}
    wb = {n: w.astype(BF16) for n, w in (("wqb", wqT), ("wkb", wkT), ("wvb", wvT))}

    # Additive diagonal masks per parity:
    # mask[m][k, q] = 0 if k + 256*m + 128*h <= q else MASK_VAL
    NM = TQ // 256
    k_idx = np.arange(P)[None, :, None]
    m_idx = np.arange(NM)[:, None, None]
    q_idx = np.arange(TQ)[None, None, :]
    masks = [
        np.where(k_idx + 256 * m_idx + P * h <= q_idx,
                 np.float32(0.0), np.float32(MASK_VAL)).astype(BF16)
        for h in (0, 1)
    ]

    in_maps = []
    for b in range(B):
        xb = x[b]                                   # [T, D] fp32
        xT = np.ascontiguousarray(xb.T)             # [D, T] fp32
        # parity gather of 128-wide key blocks
        xblk = xT.reshape(D, T // (2 * P), 2, P)    # [D, n, parity, 128]
        # tile-0 query slice: the GLOBAL first 512 queries, same on both
        # cores of the pair (no gather needed on device)
        xqb = np.ascontiguousarray(xT[:, 0:512]).astype(BF16)
        for h in (0, 1):
            xT_kv = np.ascontiguousarray(xblk[:, :, h, :].reshape(D, T // 2))
            xT_q = np.ascontiguousarray(xT[:, h * (T // 2):(h + 1) * (T // 2)])
            in_maps.append({
                "xq8": xT_q.astype(F8),
                "xqb": xqb,
                "xkv8": xT_kv.astype(F8),
                "xkvb": np.ascontiguousarray(xT_kv[:, 0:256]).astype(BF16),
                **w8, **wb,
                "mask": masks[h],
            })
    return in_maps


def _merge(results, B, T):
    """Host merge: (o0+o1)/(d0+d1)/32 per batch, back to [B,T,D] fp32."""
    out = np.empty((B, T, D), dtype=np.float32)
    for b in range(B):
        o0 = np.asarray(results[2 * b]["outT"], dtype=np.float32)
        o1 = np.asarray(results[2 * b + 1]["outT"], dtype=np.float32)
        d0 = results[2 * b]["denom"].reshape(T)
        d1 = results[2 * b + 1]["denom"].reshape(T)
        out[b] = ((o0 + o1) / ((d0 + d1) * np.float32(32.0))[None, :]).T
    return out


def kernel(x, W_q, W_k, W_v, W_o):
    from concourse.bass_utils import run_bass_kernel_spmd

    x = np.asarray(x)
    B, T, d = x.shape
    assert d == D
    TQ = 512

    key = (T, TQ)
    if key not in _PROGRAM_CACHE:
        _PROGRAM_CACHE[key] = build_program(T, TQ)
    nc = _PROGRAM_CACHE[key]

    in_maps = _prepare_core_inputs(
        np.asarray(x, np.float32), np.asarray(W_q, np.float32),
        np.asarray(W_k, np.float32), np.asarray(W_v, np.float32),
        np.asarray(W_o, np.float32), T, TQ)
    res = run_bass_kernel_spmd(nc, in_maps, list(range(2 * B)))
    return _merge(res.results, B, T)


# revision 31
# speedup vs baseline: 1.1523x; 1.1523x over previous
"""Causal self-attention (B=4, T=4096, D=1024, fp32) on 8 trn2 NeuronCores.

Sharding: 2 cores per batch. Within a batch, core h in {0,1} owns the
key blocks of parity h (128-wide blocks at global positions 2j+h). Each
core computes, for ALL queries of its batch, the unnormalized partial
attention output restricted to its own keys, plus the partial softmax
denominators. Host merge per query:
    out[q] = (o0[:,q] + o1[:,q]) / (denom0[q] + denom1[q]) / 32.

Optimizations over the plain formulation:

1. W_o folded into the V projection on the host:
   (att @ (x W_v^T)) W_o^T == att @ (x (W_o W_v)^T), so the device
   projects v' = x (W_o W_v)^T once; the attention-weighted sum of v'
   IS the (unnormalized) output. Deletes the on-device W_o GEMM (which
   was also duplicated across the pair).

2. Everything heavy runs in fp8e4 with perf_mode=DoubleRow (two
   128-deep contraction slabs per instruction = 2x the bf16 rate):
   the Q/K/V' projections (host supplies fp8 x and 32x-scaled fp8
   weights; the 32x cancels exactly: scores get 2^-15 folded into the
   Exp activation scale, v' is divided by 32 in the host merge) and the
   attention scores / att@v' / denominator GEMMs. fp32 PSUM throughout.

3. Query tile 0 (global queries 0..511) stays bf16 end to end (its own
   small bf16 projections of the first 512 queries / 256 parity keys):
   for tiny softmax supports the fp8 noise does not average out. Both
   pair cores receive the same global xqb slice, so tile-0's Q needs no
   gather and stays resident in SBUF.

4. The pair-wise Q^T AllGather is fp8 (half the bytes) and hides under
   the K/V' projection phase.

5. DMA queue discipline (head-of-line blocking is real): input loads on
   the sync queue (latency-critical x stream first, bulk weights after),
   intermediate q write-outs + collective trigger + outputs on gpsimd.
   outT is fp16 (halves output DMA bytes; the unnormalized sums fit).

Scores accumulate unscaled; softmax scale 1/sqrt(D) * 1/1024 (weight
scaling) = 2^-15 and a range bias -3.5 (keeps exp inside e4m3/fp16
range, cancels in the normalization) fold into the Exp activation. The
causal mask is additive on the raw scores, applied on PSUM before exp.
"""

import sys

if "/opt/trn_rl_repo" not in sys.path:
    sys.path.insert(0, "/opt/trn_rl_repo")

import numpy as np
import ml_dtypes

BF16 = ml_dtypes.bfloat16
F8 = ml_dtypes.float8_e4m3fn

D = 1024
P = 128          # partition / contraction block
DB = D // P      # 8 d-blocks

EXP_BIAS = -3.5      # exp(score + EXP_BIAS); cancels in the normalization
EXP_SCALE = 2.0 ** -15  # 1/sqrt(D) softmax scale * (1/32)^2 weight scaling
MASK_VAL = -4096.0 * 1024.0  # additive mask on RAW x32-scaled scores

_PROGRAM_CACHE = {}


def build_program(T, TQ):
    """Build + compile the single-core SPMD program. Returns the Bacc."""
    import concourse.mybir as mybir
    import concourse.tile as tile
    from concourse import bacc

    bf = mybir.dt.bfloat16
    f8 = mybir.dt.float8e4
    f32 = mybir.dt.float32
    f16 = mybir.dt.float16
    DR = mybir.MatmulPerfMode.DoubleRow

    NT = T // TQ             # q-tiles per core (8)
    NM = TQ // 256           # diagonal (masked) local key blocks per q-tile
    TKV = T // 2             # parity keys per core (2048)
    NKB = TKV // P           # local key blocks (16)
    KV_TT = 512              # token tile for the projection phases
    NKVT = TKV // KV_TT      # 4
    TH = T // 2              # this core's query half
    TB0 = 512                # bf16 query prefix (tile 0)
    KB0 = NM * P             # bf16 local key prefix (256)

    nc = bacc.Bacc("TRN2", target_bir_lowering=False, debug=False, num_devices=8)

    xq8 = nc.dram_tensor("xq8", [D, TH], f8, kind="ExternalInput")
    xqb = nc.dram_tensor("xqb", [D, TB0], bf, kind="ExternalInput")
    xkv8 = nc.dram_tensor("xkv8", [D, TKV], f8, kind="ExternalInput")
    xkvb = nc.dram_tensor("xkvb", [D, KB0], bf, kind="ExternalInput")
    wq8 = nc.dram_tensor("wq8", [D, D], f8, kind="ExternalInput")
    wk8 = nc.dram_tensor("wk8", [D, D], f8, kind="ExternalInput")
    wv8 = nc.dram_tensor("wv8", [D, D], f8, kind="ExternalInput")
    wqb = nc.dram_tensor("wqb", [D, D], bf, kind="ExternalInput")
    wkb = nc.dram_tensor("wkb", [D, D], bf, kind="ExternalInput")
    wvb = nc.dram_tensor("wvb", [D, D], bf, kind="ExternalInput")
    mask = nc.dram_tensor("mask", [NM, P, TQ], bf, kind="ExternalInput")
    outT = nc.dram_tensor("outT", [D, T], f16, kind="ExternalOutput")
    denom = nc.dram_tensor("denom", [NT, TQ], f32, kind="ExternalOutput")

    xq8_r = xq8.rearrange("(po pi) t -> pi po t", pi=P)
    xqb_r = xqb.rearrange("(po pi) t -> pi po t", pi=P)
    xkv8_r = xkv8.rearrange("(po pi) t -> pi po t", pi=P)
    xkvb_r = xkvb.rearrange("(po pi) t -> pi po t", pi=P)
    wq8_r = wq8.rearrange("(po pi) f -> pi po f", pi=P)
    wk8_r = wk8.rearrange("(po pi) f -> pi po f", pi=P)
    wv8_r = wv8.rearrange("(po pi) f -> pi po f", pi=P)
    wqb_r = wqb.rearrange("(po pi) f -> pi po f", pi=P)
    wkb_r = wkb.rearrange("(po pi) f -> pi po f", pi=P)
    wvb_r = wvb.rearrange("(po pi) f -> pi po f", pi=P)
    outT_r = outT.rearrange("(po pi) t -> pi po t", pi=P)

    with tile.TileContext(nc) as tc:
        with tc.tile_pool(name="res", bufs=1) as res, \
             tc.tile_pool(name="wts", bufs=1) as wts, \
             tc.tile_pool(name="dram", bufs=1, space="DRAM") as dram:
            # Persistent SBUF: fp8 K^T (d-major) and V' (token-major) for
            # the DoubleRow path; bf16 copies of the first NM key blocks
            # for the bf16 tile-0 path.
            kT8_sb = res.tile([P, DB, TKV], f8)
            v8_sb = res.tile([P, NKB, D], f8)
            kTb_sb = res.tile([P, DB, KB0], bf)
            vb_sb = res.tile([P, NM, D], bf)
            mask_sb = res.tile([P, NM, TQ], bf)
            ones_b = res.tile([P, 1], bf)
            ones_8 = res.tile([P, 2, 16], f8)
            ebias_sb = res.tile([P, 1], f32)
            nc.vector.memset(ones_b[:], 1.0)
            nc.vector.memset(ones_8[:], 1.0)
            nc.vector.memset(ebias_sb[:], EXP_BIAS)

            # Weight/x tiles all load on the sync queue; intermediate q
            # write-outs, the collective trigger and B outputs live on the
            # gpsimd queue, so sync only ever carries input loads (DMA
            # queues are head-of-line blocking).
            wq8_sb = wts.tile([P, DB, D], f8)
            wqb_sb = wts.tile([P, DB, D], bf)
            wk8_sb = wts.tile([P, DB, D], f8)
            wv8_sb = wts.tile([P, DB, D], f8)
            wkb_sb = wts.tile([P, DB, D], bf)
            wvb_sb = wts.tile([P, DB, D], bf)
            xqb_t = wts.tile([P, DB, TB0], bf)
            xkvb_t = wts.tile([P, DB, KB0], bf)
            qTb_sb = wts.tile([P, DB, TB0], bf)  # tile-0 Q^T, stays in SBUF
            # Latency-critical A0 stream goes first on sync; the bulk
            # loads for later phases are emitted after the A0-fp8 loop.
            nc.sync.dma_start(wq8_sb[:], wq8_r[:])

            # Pair-gathered fp8 Q^T. Rows [0:D] = first query half (rank
            # 2b), rows [D:2D] = second half (rank 2b+1).
            q8_local = dram.tile([D, TH], f8)
            q8_full = dram.tile([2 * D, TH], f8)

            # ---- Phase A0: Q projection of this core's query half ----
            with tc.tile_pool(name="pq_sb", bufs=2) as pq_sb, \
                 tc.tile_pool(name="pq_ps", bufs=2, space="PSUM") as pq_ps:
                # fp8 DoubleRow projection of the full query half first, so
                # the big gather fires as early as possible.
                q8_local_r = q8_local.rearrange("(po pi) t -> pi po t", pi=P)
                for it in range(TH // KV_TT):
                    xq = pq_sb.tile([P, DB, KV_TT], f8, tag="xq")
                    for po in range(DB):
                        nc.sync.dma_start(
                            xq[:, po, :],
                            xq8_r[:, po, it * KV_TT:(it + 1) * KV_TT])
                    qstage = pq_sb.tile([P, DB, KV_TT], f8, tag="qstage")
                    for do in range(DB):
                        qp = pq_ps.tile([P, KV_TT], f32, tag="qp")
                        for dd in range(DB // 2):
                            nc.tensor.matmul(
                                qp[:],
                                wq8_sb[:, 2 * dd:2 * dd + 2, do * P:(do + 1) * P],
                                xq[:, 2 * dd:2 * dd + 2, :],
                                start=(dd == 0), stop=(dd == DB // 2 - 1),
                                perf_mode=DR)
                        nc.vector.tensor_copy(qstage[:, do, :], qp[:])
                    for po in range(DB):
                        nc.gpsimd.dma_start(
                            q8_local_r[:, po, it * KV_TT:(it + 1) * KV_TT],
                            qstage[:, po, :])
                nc.gpsimd.collective_compute(
                    "AllGather",
                    mybir.AluOpType.bypass,
                    replica_groups=[[0, 1], [2, 3], [4, 5], [6, 7]],
                    ins=[q8_local[:]],
                    outs=[q8_full[:]],
                )

                # Bulk loads for the remaining phases, in first-use order,
                # queued behind the A0-fp8 x stream on sync.
                nc.sync.dma_start(wqb_sb[:], wqb_r[:])
                nc.sync.dma_start(xqb_t[:], xqb_r[:])
                nc.sync.dma_start(wkb_sb[:], wkb_r[:])
                nc.sync.dma_start(xkvb_t[:], xkvb_r[:])
                nc.sync.dma_start(wk8_sb[:], wk8_r[:])
                nc.sync.dma_start(wv8_sb[:], wv8_r[:])
                nc.sync.dma_start(wvb_sb[:], wvb_r[:])
                nc.sync.dma_start(mask_sb[:], mask.rearrange("m p t -> p m t"))

                # bf16 projection of the GLOBAL first TB0 queries (the host
                # hands both pair cores the same xqb slice, so no gather is
                # needed); result stays resident in SBUF for tile 0.
                for do in range(DB):
                    qp = pq_ps.tile([P, TB0], f32, tag="qp")
                    for di in range(DB):
                        nc.tensor.matmul(
                            qp[:],
                            wqb_sb[:, di, do * P:(do + 1) * P],
                            xqb_t[:, di, :],
                            start=(di == 0), stop=(di == DB - 1))
                    nc.vector.tensor_copy(qTb_sb[:, do, :], qp[:])

            # ---- Phase A: K/V' projection of the parity keys ----
            with tc.tile_pool(name="pa_sb", bufs=2) as pa_sb, \
                 tc.tile_pool(name="pa_ps", bufs=2, space="PSUM") as pa_ps:
                # bf16 dup of the first KB0 parity keys (tile-0 path).
                for do in range(DB):
                    kps = pa_ps.tile([P, KB0], f32, tag="kpsb")
                    for di in range(DB):
                        nc.tensor.matmul(
                            kps[:],
                            wkb_sb[:, di, do * P:(do + 1) * P],
                            xkvb_t[:, di, :],
                            start=(di == 0), stop=(di == DB - 1))
                    nc.vector.tensor_copy(kTb_sb[:, do, :], kps[:])
                for tb in range(NM):
                    for dh in range(D // 512):
                        vps = pa_ps.tile([P, 512], f32, tag="vpsb")
                        for di in range(DB):
                            nc.tensor.matmul(
                                vps[:],
                                xkvb_t[:, di, tb * P:(tb + 1) * P],
                                wvb_sb[:, di, dh * 512:(dh + 1) * 512],
                                start=(di == 0), stop=(di == DB - 1))
                        nc.vector.tensor_copy(
                            vb_sb[:, tb, dh * 512:(dh + 1) * 512], vps[:])

                # fp8 DoubleRow projections of all parity keys.
                for tt in range(NKVT):
                    xkv = pa_sb.tile([P, DB, KV_TT], f8, tag="xkv")
                    nc.sync.dma_start(
                        xkv[:], xkv8_r[:, :, tt * KV_TT:(tt + 1) * KV_TT])
                    # K^T[dout, tok] += W_k^T[din, dout].T @ x^T[din, tok]
                    for do in range(DB):
                        kps = pa_ps.tile([P, KV_TT], f32, tag="kps")
                        for dd in range(DB // 2):
                            nc.tensor.matmul(
                                kps[:],
                                wk8_sb[:, 2 * dd:2 * dd + 2, do * P:(do + 1) * P],
                                xkv[:, 2 * dd:2 * dd + 2, :],
                                start=(dd == 0), stop=(dd == DB // 2 - 1),
                                perf_mode=DR)
                        nc.vector.tensor_copy(
                            kT8_sb[:, do, tt * KV_TT:(tt + 1) * KV_TT], kps[:])
                    # V'[tok, dout] += x^T[din, tok].T @ W_vo^T[din, dout]
                    for tb in range(KV_TT // P):
                        for dh in range(D // 512):
                            vps = pa_ps.tile([P, 512], f32, tag="vps")
                            for dd in range(DB // 2):
                                nc.tensor.matmul(
                                    vps[:],
                                    xkv[:, 2 * dd:2 * dd + 2,
                                        tb * P:(tb + 1) * P],
                                    wv8_sb[:, 2 * dd:2 * dd + 2,
                                           dh * 512:(dh + 1) * 512],
                                    start=(dd == 0), stop=(dd == DB // 2 - 1),
                                    perf_mode=DR)
                            jb = tt * (KV_TT // P) + tb
                            nc.vector.tensor_copy(
                                v8_sb[:, jb, dh * 512:(dh + 1) * 512], vps[:])

            # ---- Phase B: per q-tile attention (already the output) ----
            q8_full_r = q8_full.rearrange("(ho po pi) t -> pi ho po t",
                                          pi=P, po=DB)
            with tc.tile_pool(name="pb_sb", bufs=2) as pb_sb, \
                 tc.tile_pool(name="pb_pan", bufs=2) as pb_pan, \
                 tc.tile_pool(name="pb_out", bufs=4) as pb_out, \
                 tc.tile_pool(name="s_ps", bufs=3, space="PSUM") as s_ps, \
                 tc.tile_pool(name="y_ps", bufs=3, space="PSUM") as y_ps, \
                 tc.tile_pool(name="d_ps", bufs=1, space="PSUM") as d_ps:
                for i in range(NT):
                    nkb = (i + 1) * NM  # local key blocks for this q-tile
                    q0 = i * TQ
                    ho = q0 // TH       # which gathered half holds this tile
                    qh = q0 - ho * TH
                    f8_tile = i > 0     # tile 0 stays bf16

                    if f8_tile:
                        qT8 = pb_sb.tile([P, DB, TQ], f8, tag="qT8")
                        for po in range(DB):
                            nc.sync.dma_start(
                                qT8[:, po, :],
                                q8_full_r[:, ho, po, qh:qh + TQ])
                    else:
                        qTb = qTb_sb  # resident since phase A0

                    # S^T blocks -> mask -> exp -> panel
                    if f8_tile:
                        panel = pb_pan.tile([P, NT * NM, TQ], f8, tag="pan8")
                    else:
                        panel = pb_pan.tile([P, NM, TQ], bf, tag="panb")
                    for j in range(nkb):
                        sps = s_ps.tile([P, TQ], f32, tag="s")
                        if f8_tile:
                            for dd in range(DB // 2):
                                nc.tensor.matmul(
                                    sps[:],
                                    kT8_sb[:, 2 * dd:2 * dd + 2,
                                           j * P:(j + 1) * P],
                                    qT8[:, 2 * dd:2 * dd + 2, :],
                                    start=(dd == 0), stop=(dd == DB // 2 - 1),
                                    perf_mode=DR)
                        else:
                            for di in range(DB):
                                nc.tensor.matmul(
                                    sps[:],
                                    kTb_sb[:, di, j * P:(j + 1) * P],
                                    qTb[:, di, :],
                                    start=(di == 0), stop=(di == DB - 1))
                        if j >= nkb - NM:
                            m = j - (nkb - NM)
                            nc.vector.tensor_add(
                                out=sps[:], in0=sps[:], in1=mask_sb[:, m, :])
                        nc.scalar.activation(
                            panel[:, j, :], sps[:],
                            mybir.ActivationFunctionType.Exp,
                            bias=ebias_sb[:], scale=EXP_SCALE)

                    # Denominators after all panels (keeps the tensor queue
                    # from stalling on the exp producer mid-stream).
                    dps = d_ps.tile([1, TQ], f32, tag="den")
                    if f8_tile:
                        for jj in range(nkb // 2):
                            nc.tensor.matmul(
                                dps[:], ones_8[:, :, 0:1],
                                panel[:, 2 * jj:2 * jj + 2, :],
                                start=(jj == 0), stop=(jj == nkb // 2 - 1),
                                perf_mode=DR)
                    else:
                        for j in range(nkb):
                            nc.tensor.matmul(
                                dps[:], ones_b[:], panel[:, j, :],
                                start=(j == 0), stop=(j == nkb - 1))
                    dstage = pb_sb.tile([1, TQ], f32, tag="dstage")
                    nc.vector.tensor_copy(dstage[:], dps[:])
                    nc.gpsimd.dma_start(denom[i:i + 1, :], dstage[0:1, :])

                    # outT[dout, q] += V'[k, dout].T @ expS^T[k, q]
                    for do in range(DB):
                        yps = y_ps.tile([P, TQ], f32, tag="y")
                        if f8_tile:
                            for jj in range(nkb // 2):
                                nc.tensor.matmul(
                                    yps[:],
                                    v8_sb[:, 2 * jj:2 * jj + 2,
                                          do * P:(do + 1) * P],
                                    panel[:, 2 * jj:2 * jj + 2, :],
                                    start=(jj == 0), stop=(jj == nkb // 2 - 1),
                                    perf_mode=DR)
                        else:
                            for j in range(nkb):
                                nc.tensor.matmul(
                                    yps[:],
                                    vb_sb[:, j, do * P:(do + 1) * P],
                                    panel[:, j, :],
                                    start=(j == 0), stop=(j == nkb - 1))
                        ostage = pb_out.tile([P, TQ], f16, tag="ostage")
                        nc.vector.tensor_copy(ostage[:], yps[:])
                        nc.gpsimd.dma_start(
                            outT_r[:, do, q0:q0 + TQ], ostage[:])

    nc.compile()
    return nc


def _prepare_core_inputs(x, W_q, W_k, W_v, W_o, T, TQ):
    """Host-side shard prep. Returns list of 8 in_maps."""
    B = x.shape[0]

    # Fold W_o into the V projection; scale all weights by 32 so their
    # fp8 casts stay out of e4m3's subnormal range (exactly compensated
    # via EXP_SCALE and the /32 in the merge).
    w_vo = np.asarray(W_o, np.float32) @ np.asarray(W_v, np.float32)
    wqT = np.ascontiguousarray(W_q.T) * np.float32(32.0)
    wkT = np.ascontiguousarray(W_k.T) * np.float32(32.0)
    wvT = np.ascontiguousarray(w_vo.T) * np.float32(32.0)
    w8 = {n: w.astype(F8) for n, w in (("wq8", wqT), ("wk8", wkT), ("wv8", wvT))}
    wb = {n: w.astype(BF16) for n, w in (("wqb", wqT), ("wkb", wkT), ("wvb", wvT))}

    # Additive diagonal masks per parity:
    # mask[m][k, q] = 0 if k + 256*m + 128*h <= q else MASK_VAL
    NM = TQ // 256
    k_idx = np.arange(P)[None, :, None]
    m_idx = np.arange(NM)[:, None, None]
    q_idx = np.arange(TQ)[None, None, :]
    masks = [
        np.where(k_idx + 256 * m_idx + P * h <= q_idx,
                 np.float32(0.0), np.float32(MASK_VAL)).astype(BF16)
        for h in (0, 1)
    ]

    in_maps = []
    for b in range(B):
        xb = x[b]                                   # [T, D] fp32
        xT = np.ascontiguousarray(xb.T)             # [D, T] fp32
        # parity gather of 128-wide key blocks
        xblk = xT.reshape(D, T // (2 * P), 2, P)    # [D, n, parity, 128]
        # tile-0 query slice: the GLOBAL first 512 queries, same on both
        # cores of the pair (no gather needed on device)
        xqb = np.ascontiguousarray(xT[:, 0:512]).astype(BF16)
        for h in (0, 1):
            xT_kv = np.ascontiguousarray(xblk[:, :, h, :].reshape(D, T // 2))
            xT_q = np.ascontiguousarray(xT[:, h * (T // 2):(h + 1) * (T // 2)])
            in_maps.append({
                "xq8": xT_q.astype(F8),
                "xqb": xqb,
                "xkv8": xT_kv.astype(F8),
                "xkvb": np.ascontiguousarray(xT_kv[:, 0:256]).astype(BF16),
                **w8, **wb,
                "mask": masks[h],
            })
    return in_maps


def _merge(results, B, T):
    """Host merge: (o0+o1)/(d0+d1)/32 per batch, back to [B,T,D] fp32."""
    out = np.empty((B, T, D), dtype=np.float32)
    for b in range(B):
        o0 = np.asarray(results[2 * b]["outT"], dtype=np.float32)
        o1 = np.asarray(results[2 * b + 1]["outT"], dtype=np.float32)
        d0 = results[2 * b]["denom"].reshape(T)
        d1 = results[2 * b + 1]["denom"].reshape(T)
        out[b] = ((o0 + o1) / ((d0 + d1) * np.float32(32.0))[None, :]).T
    return out


def kernel(x, W_q, W_k, W_v, W_o):
    from concourse.bass_utils import run_bass_kernel_spmd

    x = np.asarray(x)
    B, T, d = x.shape
    assert d == D
    TQ = 512

    key = (T, TQ)
    if key not in _PROGRAM_CACHE:
        _PROGRAM_CACHE[key] = build_program(T, TQ)
    nc = _PROGRAM_CACHE[key]

    in_maps = _prepare_core_inputs(
        np.asarray(x, np.float32), np.asarray(W_q, np.float32),
        np.asarray(W_k, np.float32), np.asarray(W_v, np.float32),
        np.asarray(W_o, np.float32), T, TQ)
    res = run_bass_kernel_spmd(nc, in_maps, list(range(2 * B)))
    return _merge(res.results, B, T)
